# revision 1
# baseline (speedup 1.0000x reference)
"""Trainium2 Bass kernel for a 3-layer GraphConv GNN (N=100k, E=1.6M, F=128).

Strategy (8 NeuronCores):
- Nodes sharded by dst across cores (12500/core, padded to 12544 = 98 blocks
  of 128). Edges partitioned by dst owner so aggregation is core-local.
- Aggregation: per 128-edge chunk, gather source rows (dma_gather, int16
  indices bucketed into <=25088-row ranges of the table) and scatter-add via a
  one-hot selection matmul into PSUM (S[e,d] = (dst_local==d) * norm_dst).
- Feature-major pipeline: psum_agg[f,d] -> W matmul -> relu+bias -> transpose
  -> *norm_src -> per-core table slice; AllGather slices into the full
  node-major table for the next layer's gathers.
- Final: fused [fc_W|attn_W] matmul, sigmoid gate, bias, softmax on-device.
"""
import os
import sys

sys.path.insert(0, "/opt/trn_rl_repo")

import numpy as np
import ml_dtypes

N = 100000
E = 1600000
F = 128
NCLS = 8
NCORES = 8
NPC = 12500          # nodes per core
PADN = 12544         # padded nodes per core (98 * 128)
NB = 98              # dst blocks per core
TBL = PADN * NCORES  # table rows in AllGather layout (100352)
NBKT = 4
QW = TBL // NBKT     # bucket width 25088 (< 32768 so int16 local idx works)
G = 7                # blocks per group
NGRP = NB // G       # 14 groups

USE_BF16 = os.environ.get("GNN_F32", "0") != "1"
LAYERS = int(os.environ.get("GNN_LAYERS", "3"))
SKIP_AG = os.environ.get("GNN_SKIP_AG", "0") == "1"

_CACHE = {}


def _host_schedule(src, dst):
    """Partition/sort edges; emit per-core gather/scatter schedule arrays."""
    src = np.asarray(src, dtype=np.int64)
    dst = np.asarray(dst, dtype=np.int64)

    deg_out = np.bincount(src, minlength=N).astype(np.float32)
    deg_in = np.bincount(dst, minlength=N).astype(np.float32)
    norm_src = np.where(deg_out > 0, 1.0 / np.sqrt(np.maximum(deg_out, 1.0)), 0.0).astype(np.float32)
    norm_dst = np.where(deg_in > 0, 1.0 / np.sqrt(np.maximum(deg_in, 1.0)), 0.0).astype(np.float32)

    rsrc = (src // NPC) * PADN + (src % NPC)  # remapped to AG table layout
    owner = dst // NPC

    per_core = []
    cnt_all = np.zeros((NCORES, NB, NBKT), dtype=np.int64)
    for c in range(NCORES):
        sel = owner == c
        es = rsrc[sel]
        ed = dst[sel] - c * NPC
        nd = norm_dst[dst[sel]]
        blk = ed >> 7
        dloc = (ed & 127).astype(np.float32)
        bkt = es // QW
        key = blk * NBKT + bkt
        order = np.argsort(key, kind="stable")
        es, dloc, nd, key = es[order], dloc[order], nd[order], key[order]
        cnt = np.bincount(key, minlength=NB * NBKT).reshape(NB, NBKT)
        cnt_all[c] = cnt
        per_core.append((es, dloc, nd, cnt))

    C = np.ceil(cnt_all.max(axis=0) / 128.0).astype(np.int64)  # [NB, NBKT] chunk capacities
    T = int(C.sum())

    # canonical chunk order: group g -> bucket k -> block b in group -> chunk j
    chunk_start = np.zeros((NB, NBKT), dtype=np.int64)  # global chunk index of (b,k)
    q = 0
    for g in range(NGRP):
        for k in range(NBKT):
            for b in range(g * G, (g + 1) * G):
                chunk_start[b, k] = q
                q += C[b, k]
    assert q == T

    cores = []
    for c in range(NCORES):
        es, dloc, nd, cnt = per_core[c]
        off = np.zeros(NB * NBKT + 1, dtype=np.int64)
        np.cumsum(cnt.reshape(-1), out=off[1:])
        idx_flat = np.zeros(T * 128, dtype=np.int16)
        dstl_flat = np.full(T * 128, 999.0, dtype=np.float32)
        enorm_flat = np.zeros(T * 128, dtype=np.float32)
        for b in range(NB):
            for k in range(NBKT):
                n = cnt[b, k]
                if n == 0:
                    continue
                s0 = off[b * NBKT + k]
                p0 = chunk_start[b, k] * 128
                idx_flat[p0:p0 + n] = (es[s0:s0 + n] - k * QW).astype(np.int16)
                dstl_flat[p0:p0 + n] = dloc[s0:s0 + n]
                enorm_flat[p0:p0 + n] = nd[s0:s0 + n]
        # wrap idx per (g,k) gather segment: [16, n/16], idx i at [i%16, i//16]
        idx_w = np.zeros((16, T * 8), dtype=np.int16)
        for g in range(NGRP):
            for k in range(NBKT):
                b0 = g * G
                q0 = int(chunk_start[b0, k])
                nch = int(C[b0:b0 + G, k].sum())
                if nch == 0:
                    continue
                seg = idx_flat[q0 * 128:(q0 + nch) * 128]
                idx_w[:, q0 * 8:(q0 + nch) * 8] = seg.reshape(-1, 16).T
        cores.append({
            "idx16": np.tile(idx_w, (8, 1)),
            "dstl": np.ascontiguousarray(dstl_flat.reshape(T, 128).T),
            "enorm": np.ascontiguousarray(enorm_flat.reshape(T, 128).T),
        })
    return C, T, chunk_start, cores, norm_src, norm_dst


def _build_nc(C, T, chunk_start, attn_b_val):
    import concourse.mybir as mybir
    import concourse.bacc as bacc
    import concourse.tile as tile
    from concourse.masks import make_identity

    DT = mybir.dt.bfloat16 if USE_BF16 else mybir.dt.float32
    f32 = mybir.dt.float32

    nc = bacc.Bacc("TRN2", target_bir_lowering=False, debug=False, num_devices=NCORES)
    t1_d = nc.dram_tensor("t1", [TBL, F], DT, kind="ExternalInput")
    idx_d = nc.dram_tensor("idx16", [128, T * 8], mybir.dt.int16, kind="ExternalInput")
    dstl_d = nc.dram_tensor("dstl", [128, T], f32, kind="ExternalInput")
    enorm_d = nc.dram_tensor("enorm", [128, T], f32, kind="ExternalInput")
    ns_d = nc.dram_tensor("nsb", [128, NB], f32, kind="ExternalInput")
    w_d = [nc.dram_tensor(f"w{i}", [F, F], DT, kind="ExternalInput") for i in (1, 2, 3)]
    b_d = [nc.dram_tensor(f"b{i}", [F, 1], f32, kind="ExternalInput") for i in (1, 2, 3)]
    fca_d = nc.dram_tensor("fca", [F, NCLS + 1], DT, kind="ExternalInput")
    fcb_d = nc.dram_tensor("fcb", [128, NCLS], f32, kind="ExternalInput")
    probs_d = nc.dram_tensor("probs", [PADN, NCLS], f32, kind="ExternalOutput")

    max_chunks_gk = 0
    for g in range(NGRP):
        for k in range(NBKT):
            max_chunks_gk = max(max_chunks_gk, int(C[g * G:(g + 1) * G, k].sum()))

    with tile.TileContext(nc) as tc:
        with tc.tile_pool(name="const", bufs=1) as cpool, \
             tc.tile_pool(name="msgp", bufs=8 if USE_BF16 else 4) as msgp, \
             tc.tile_pool(name="sp", bufs=8) as spool, \
             tc.tile_pool(name="wk", bufs=3) as wk, \
             tc.tile_pool(name="pagg", bufs=2, space="PSUM") as pagg, \
             tc.tile_pool(name="ph", bufs=2, space="PSUM") as ph, \
             tc.tile_pool(name="pt", bufs=2, space="PSUM") as pt, \
             tc.tile_pool(name="pm", bufs=2, space="PSUM") as pm, \
             tc.tile_pool(name="dram", bufs=1, space="DRAM") as dram:

            # constants
            iota_i = cpool.tile([128, 128], mybir.dt.int32)
            nc.gpsimd.iota(iota_i[:], pattern=[[1, 128]], base=0, channel_multiplier=0)
            iota_dt = cpool.tile([128, 128], DT)
            nc.vector.tensor_copy(out=iota_dt[:], in_=iota_i[:])
            ident = cpool.tile([128, 128], DT)
            make_identity(nc, ident[:])

            idx_t = cpool.tile([128, T * 8], mybir.dt.int16)
            nc.sync.dma_start(out=idx_t[:], in_=idx_d.ap())
            dstl_t = cpool.tile([128, T], f32)
            nc.sync.dma_start(out=dstl_t[:], in_=dstl_d.ap())
            enorm_t = cpool.tile([128, T], f32)
            nc.sync.dma_start(out=enorm_t[:], in_=enorm_d.ap())
            ns_t = cpool.tile([128, NB], f32)
            nc.sync.dma_start(out=ns_t[:], in_=ns_d.ap())
            w_t = []
            b_t = []
            for i in range(3):
                wt = cpool.tile([F, F], DT, tag=f"w{i}")
                nc.sync.dma_start(out=wt[:], in_=w_d[i].ap())
                w_t.append(wt)
                bt = cpool.tile([F, 1], f32, tag=f"b{i}")
                nc.sync.dma_start(out=bt[:], in_=b_d[i].ap())
                b_t.append(bt)
            fca_t = cpool.tile([F, NCLS + 1], DT)
            nc.sync.dma_start(out=fca_t[:], in_=fca_d.ap())
            fcb_t = cpool.tile([128, NCLS], f32)
            nc.sync.dma_start(out=fcb_t[:], in_=fcb_d.ap())

            # inter-layer tables
            tables = [t1_d.ap()]
            ccins = []
            for l in (2, 3):
                tbl = dram.tile([TBL, F], DT, tag=f"tbl{l}", addr_space="Shared")
                cci = dram.tile([PADN, F], DT, tag=f"cci{l}")
                tables.append(tbl[:])
                ccins.append(cci)

            for l in range(LAYERS):
                table_ap = tables[l]
                for g in range(NGRP):
                    msgs = {}
                    for k in range(NBKT):
                        nch = int(C[g * G:(g + 1) * G, k].sum())
                        if nch == 0:
                            continue
                        q0 = int(chunk_start[g * G, k])
                        m = msgp.tile([128, nch, F], DT, tag="msg")
                        nc.gpsimd.dma_gather(
                            m[:], table_ap[k * QW:TBL, :],
                            idx_t[:, q0 * 8:(q0 + nch) * 8],
                            nch * 128, nch * 128, F, single_packet=False)
                        msgs[k] = (m, q0)
                    for b in range(g * G, (g + 1) * G):
                        nch_b = int(C[b].sum())
                        ps = pagg.tile([128, 128], f32, tag="pagg")
                        ci = 0
                        for k in range(NBKT):
                            for j in range(int(C[b, k])):
                                m, q0 = msgs[k]
                                col = int(chunk_start[b, k]) + j
                                s_t = spool.tile([128, 128], DT, tag="s")
                                nc.vector.tensor_scalar(
                                    out=s_t[:], in0=iota_dt[:],
                                    scalar1=dstl_t[:, col:col + 1],
                                    scalar2=enorm_t[:, col:col + 1],
                                    op0=mybir.AluOpType.is_equal,
                                    op1=mybir.AluOpType.mult)
                                nc.tensor.matmul(
                                    out=ps[:], lhsT=m[:, col - q0, :], rhs=s_t[:],
                                    start=(ci == 0), stop=(ci == nch_b - 1))
                                ci += 1
                        aggT = wk.tile([128, 128], DT, tag="aggT")
                        nc.vector.tensor_copy(out=aggT[:], in_=ps[:])
                        psh = ph.tile([128, 128], f32, tag="ph")
                        nc.tensor.matmul(out=psh[:], lhsT=w_t[l][:], rhs=aggT[:],
                                         start=True, stop=True)
                        h_sb = wk.tile([128, 128], DT, tag="h")
                        nc.scalar.activation(h_sb[:], psh[:],
                                             mybir.ActivationFunctionType.Relu,
                                             bias=b_t[l][:, :1], scale=1.0)
                        if l < LAYERS - 1:
                            pst = pt.tile([128, 128], DT, tag="pt")
                            nc.tensor.transpose(out=pst[:], in_=h_sb[:], identity=ident[:])
                            xt = wk.tile([128, 128], DT, tag="xt")
                            nc.vector.tensor_scalar(
                                out=xt[:], in0=pst[:], scalar1=ns_t[:, b:b + 1],
                                scalar2=None, op0=mybir.AluOpType.mult)
                            nc.sync.dma_start(
                                out=ccins[l][b * 128:(b + 1) * 128, :], in_=xt[:])
                        else:
                            pla = pm.tile([128, NCLS + 1], f32, tag="pla")
                            nc.tensor.matmul(out=pla[:], lhsT=h_sb[:], rhs=fca_t[:],
                                             start=True, stop=True)
                            attn = wk.tile([128, 1], f32, tag="attn")
                            nc.scalar.activation(attn[:], pla[:, NCLS:NCLS + 1],
                                                 mybir.ActivationFunctionType.Sigmoid,
                                                 bias=float(attn_b_val), scale=1.0)
                            logits = wk.tile([128, NCLS], f32, tag="logits")
                            nc.vector.tensor_scalar(
                                out=logits[:], in0=pla[:, :NCLS], scalar1=attn[:, :1],
                                scalar2=None, op0=mybir.AluOpType.mult)
                            nc.vector.tensor_tensor(
                                out=logits[:], in0=logits[:], in1=fcb_t[:],
                                op=mybir.AluOpType.add)
                            mx = wk.tile([128, 1], f32, tag="mx")
                            nc.vector.tensor_reduce(
                                out=mx[:], in_=logits[:], axis=mybir.AxisListType.X,
                                op=mybir.AluOpType.max)
                            sh = wk.tile([128, NCLS], f32, tag="sh")
                            nc.vector.tensor_scalar(
                                out=sh[:], in0=logits[:], scalar1=mx[:, :1],
                                scalar2=None, op0=mybir.AluOpType.subtract)
                            ex = wk.tile([128, NCLS], f32, tag="ex")
                            ssum = wk.tile([128, 1], f32, tag="ssum")
                            nc.scalar.activation(ex[:], sh[:],
                                                 mybir.ActivationFunctionType.Exp,
                                                 accum_out=ssum[:, :1])
                            rinv = wk.tile([128, 1], f32, tag="rinv")
                            nc.vector.reciprocal(rinv[:, :1], ssum[:, :1])
                            pr = wk.tile([128, NCLS], f32, tag="pr")
                            nc.vector.tensor_scalar(
                                out=pr[:], in0=ex[:], scalar1=rinv[:, :1],
                                scalar2=None, op0=mybir.AluOpType.mult)
                            nc.sync.dma_start(
                                out=probs_d.ap()[b * 128:(b + 1) * 128, :], in_=pr[:])
                if l < LAYERS - 1 and not SKIP_AG:
                    nc.gpsimd.collective_compute(
                        "AllGather", mybir.AluOpType.bypass,
                        replica_groups=[list(range(NCORES))],
                        ins=[ccins[l].opt()], outs=[tables[l + 1].tensor.ap()])
    nc.compile()
    return nc


def _prepare(inputs):
    src = inputs["src"]
    dst = inputs["dst"]
    key = (src.tobytes(), dst.tobytes())
    C, T, chunk_start, cores, norm_src, norm_dst = _host_schedule(src, dst)

    np_dt = ml_dtypes.bfloat16 if USE_BF16 else np.float32

    feats = np.asarray(inputs["features"], dtype=np.float32)
    xt1 = feats * norm_src[:, None]
    t1 = np.zeros((TBL, F), dtype=np_dt)
    for c in range(NCORES):
        t1[c * PADN:c * PADN + NPC] = xt1[c * NPC:(c + 1) * NPC].astype(np_dt)

    fca = np.concatenate([np.asarray(inputs["fc_W"], np.float32),
                          np.asarray(inputs["attn_W"], np.float32)], axis=1).astype(np_dt)
    fcb = np.tile(np.asarray(inputs["fc_b"], np.float32)[None, :], (128, 1))

    in_maps = []
    for c in range(NCORES):
        ns_col = np.zeros((128, NB), dtype=np.float32)
        loc = np.arange(PADN)
        valid = loc < NPC
        vals = np.zeros(PADN, dtype=np.float32)
        vals[valid] = norm_src[c * NPC + loc[valid]]
        ns_col[:, :] = vals.reshape(NB, 128).T
        m = {
            "t1": t1,
            "idx16": cores[c]["idx16"],
            "dstl": cores[c]["dstl"],
            "enorm": cores[c]["enorm"],
            "nsb": ns_col,
            "fca": fca,
            "fcb": fcb.astype(np.float32),
        }
        for i, wn in enumerate(("W1", "W2", "W3")):
            m[f"w{i + 1}"] = np.asarray(inputs[wn], np.float32).astype(np_dt)
        for i, bn in enumerate(("b1", "b2", "b3")):
            m[f"b{i + 1}"] = np.asarray(inputs[bn], np.float32).reshape(F, 1)
        in_maps.append(m)

    attn_b_val = float(np.asarray(inputs["attn_b"]).reshape(-1)[0])
    return (C, T, chunk_start, attn_b_val), in_maps


def run(inputs, trace=False):
    from concourse.bass_utils import run_bass_kernel_spmd

    (C, T, chunk_start, attn_b_val), in_maps = _prepare(inputs)
    ck = ("nc", C.tobytes(), T, USE_BF16, attn_b_val, LAYERS, SKIP_AG)
    if ck not in _CACHE:
        _CACHE[ck] = _build_nc(C, T, chunk_start, attn_b_val)
    nc = _CACHE[ck]
    try:
        res = run_bass_kernel_spmd(nc, in_maps, core_ids=list(range(NCORES)), trace=trace)
    except ModuleNotFoundError:
        res = run_bass_kernel_spmd(nc, in_maps, core_ids=list(range(NCORES)), trace=False)
    out = np.empty((N, NCLS), dtype=np.float32)
    for c in range(NCORES):
        out[c * NPC:(c + 1) * NPC] = res.results[c]["probs"][:NPC]
    return out, res


def kernel(**inputs):
    return run(inputs)[0]

